# revision 1
# baseline (speedup 1.0000x reference)
"""Block-sparse attention on 8 Trainium2 NeuronCores (Bass/Tile SPMD kernel).

Sharding: batch*head_groups across the 8 cores. Core c handles batch c//4 and
heads [4*(c%4), 4*(c%4)+4). Projection weights are sliced per core host-side
(pre-transposed + bf16-cast); the [16,16] block mask specializes the compiled
program (only kept blocks are computed). Each core emits a partial output
(its 256-wide d-slice pushed through Wo); the host sums the 4 partials per
batch and adds the bias.

Layout strategy per core (all attention math in "transposed" orientation):
  - x^T [1024, 2048] bf16 resident in SBUF (8 partition tiles)
  - q^T, k^T computed as [256, 2048] (2 pair-tiles of 128 partitions: each
    pair-tile stacks 2 heads of 64 rows); v stored as [v_h|1] blocks of 65
    cols per head so one M=65 matmul yields out2^T rows AND the softmax
    denominator row
  - attention sweeps k-blocks (j) outer within half-row chunks of 8 query
    blocks: the k^T_j stationary is loaded once per (head, j) and reused
    across that chunk's kept query blocks, with consecutive kept blocks
    fused into single wide-N matmuls (split only at PSUM bank boundaries)
  - one exp per (j, chunk) covers both heads' scores ([128, <=2048] PSUM)
  - normalize via reciprocal of the denominator row + DMA partition
    broadcast + two DVE multiplies into the bf16 out^T pair tile
  - final: out_partial[s, :] accumulates outTbf[p].T @ woT[p] over 2 pairs
"""

import time
from contextlib import ExitStack

import ml_dtypes
import numpy as np

import concourse.bass as bass
import concourse.tile as tile
from concourse import bacc, mybir
from concourse.ap import AP as APClass
from concourse.bass_utils import run_bass_kernel_spmd

BF16 = mybir.dt.bfloat16
F32 = mybir.dt.float32
bf16 = ml_dtypes.bfloat16

B, S, D, H = 2, 2048, 1024, 16
DH = 64
BLK = 128
NB = 16
NCORES = 8
HPC = H // (NCORES // B)   # 4 heads per core
E = HPC * DH               # 256 projection columns per core
EV = HPC * (DH + 1)        # 260: v stored as [v_h | ones] per head
KD = D // 128              # 8 contraction chunks

ALL_PHASES = ("proj", "attn", "final")

_nc_cache: dict = {}
last_run_info: dict = {}


def _bcast_ap(sl, n):
    """[1, W] SBUF slice -> [1, n, W] AP replicating the row n times (for DMA)."""
    apl = [list(x) for x in sl.ap]
    assert len(apl) == 2 and apl[0][1] == 1, apl
    return APClass(sl.tensor, sl.offset, [apl[0], [0, n], apl[1]])


def _runs_of(lst):
    out = []
    for i in lst:
        if out and i == out[-1][-1] + 1:
            out[-1].append(i)
        else:
            out.append([i])
    return out


def _emit(tc, aps, kept, loop_n=0, phases=ALL_PHASES):
    if loop_n:
        # measurement mode: run the whole body loop_n times on-device so the
        # per-iteration kernel time can be extracted from wall-clock diffs
        with tc.For_i(0, loop_n, 1):
            _emit_body(tc, aps, kept, phases)
    else:
        _emit_body(tc, aps, kept, phases)


def _emit_body(tc, aps, kept, phases=ALL_PHASES):
    nc = tc.nc
    xT_ap, wqT_ap, wkT_ap, wvT_ap, woT_ap, outp_ap = aps
    Exp = mybir.ActivationFunctionType.Exp

    with ExitStack() as ctx:
        persist = ctx.enter_context(tc.tile_pool(name="persist", bufs=1))

        # ---- Phase 0: load inputs -------------------------------------------------
        xT = []
        for kd in range(KD):
            t = persist.tile([128, S], BF16, name=f"xT{kd}", tag=f"xT{kd}")
            nc.sync.dma_start(t[:], xT_ap[kd * 128:(kd + 1) * 128, :])
            xT.append(t)

        def load_w(src_ap, name):
            ts = []
            for kd in range(KD):
                t = persist.tile([128, E], BF16, name=f"{name}{kd}", tag=f"{name}{kd}")
                nc.sync.dma_start(t[:], src_ap[kd * 128:(kd + 1) * 128, :])
                ts.append(t)
            return ts

        wq = load_w(wqT_ap, "wq")
        wk = load_w(wkT_ap, "wk")
        wv = load_w(wvT_ap, "wv")
        wo = []
        for p in range(2):
            t = persist.tile([128, D], BF16, name=f"wo{p}", tag=f"wo{p}")
            nc.sync.dma_start(t[:], woT_ap[p * 128:(p + 1) * 128, :])
            wo.append(t)

        zeros_sb = persist.tile([128, 512], BF16, name="zeros_sb", tag="zeros_sb")
        nc.any.memset(zeros_sb[:], 0.0)
        qT = [persist.tile([128, S], BF16, name=f"qT{p}", tag=f"qT{p}") for p in range(2)]
        kT = [persist.tile([128, S], BF16, name=f"kT{p}", tag=f"kT{p}") for p in range(2)]
        # v tiles: per s-block, layout [v_h0 | 1 | v_h1 | 1 | v_h2 | 1 | v_h3 | 1]
        vv = [persist.tile([128, EV], BF16, name=f"v{m}", tag=f"v{m}") for m in range(S // 128)]
        outTbf = [persist.tile([128, S], BF16, name=f"oT{p}", tag=f"oT{p}") for p in range(2)]

        if "proj" not in phases:
            return
        # ---- Phase 1: projections -------------------------------------------------
        with tc.tile_pool(name="proj_ps", bufs=8, space="PSUM") as proj_ps:
            # q^T and k^T: stationary = weight chunk, moving = x^T s-chunks
            for dst, w in ((qT, wq), (kT, wk)):
                for p in range(2):
                    pss = [proj_ps.tile([128, 512], F32, name="projps", tag="proj") for _ in range(4)]
                    for kd in range(KD):
                        for sc in range(4):
                            nc.tensor.matmul(
                                pss[sc][:],
                                w[kd][:, p * 128:(p + 1) * 128],
                                xT[kd][:, sc * 512:(sc + 1) * 512],
                                start=(kd == 0),
                                stop=(kd == KD - 1),
                            )
                    for sc in range(4):
                        nc.vector.tensor_copy(dst[p][:, sc * 512:(sc + 1) * 512], pss[sc][:])
            # v natural: stationary = x^T s-tile chunk, moving = wv
            for m in range(S // 128):
                ps = proj_ps.tile([128, 512], F32, name="projv", tag="proj")
                for kd in range(KD):
                    nc.tensor.matmul(
                        ps[:, 0:E],
                        xT[kd][:, m * 128:(m + 1) * 128],
                        wv[kd][:],
                        start=(kd == 0),
                        stop=(kd == KD - 1),
                    )
                # ones columns at 64, 129, 194, 259; v data strided around them
                v3 = vv[m].rearrange("p (g c) -> p g c", g=HPC)
                nc.any.memset(v3[:, :, 64:65], 1.0)
                nc.vector.tensor_copy(
                    v3[:, :, 0:64],
                    ps[:, 0:E].rearrange("p (g c) -> p g c", g=HPC),
                )

        if "attn" not in phases:
            # keep outputs defined for the runner
            nc.any.memset(outTbf[0][:, 0:128], 0.0)
            return
        # ---- Phase 2: block-sparse attention -------------------------------------
        # kept[i] = kept k-blocks (j) for query block i
        col_kept = [[i for i in range(NB) if j in kept[i]] for j in range(NB)]
        first_j = {i: kept[i][0] for i in range(NB)}
        last_j = {i: kept[i][-1] for i in range(NB)}

        with ExitStack() as actx:
            scA_pool = actx.enter_context(tc.tile_pool(name="scA_ps", bufs=1, space="PSUM"))
            scB_pool = actx.enter_context(tc.tile_pool(name="scB_ps", bufs=1, space="PSUM"))
            avA_pool = actx.enter_context(tc.tile_pool(name="avA_ps", bufs=1, space="PSUM"))
            avB_pool = actx.enter_context(tc.tile_pool(name="avB_ps", bufs=1, space="PSUM"))
            attn_pool = actx.enter_context(tc.tile_pool(name="attn_sb", bufs=4))
            norm_pool = actx.enter_context(tc.tile_pool(name="norm_sb", bufs=6))
            FILL = 1024  # per-head score fill width (2 PSUM banks)

            for p in range(2):
                for ch in range(2):
                    irange = list(range(ch * 8, ch * 8 + 8))
                    avA = [avA_pool.tile([65, 512], F32, name=f"avA{g}", tag=f"avA{g}")
                           for g in range(2)]
                    avB = [avB_pool.tile([65, 512], F32, name=f"avB{g}", tag=f"avB{g}")
                           for g in range(2)]
                    # zero-prime each av bank: one start=True matmul covering the
                    # full bank, so all region accumulations can use start=False
                    for t in avA + avB:
                        nc.tensor.matmul(t[:], vv[0][:, 0:65], zeros_sb[:],
                                         start=True, stop=False, skip_group_check=True)

                    def av_region(tiles, i):
                        li = i - ch * 8
                        return tiles[li // 4][:, (li % 4) * 128:(li % 4) * 128 + 128]

                    def flush(fill_js, scA, scB, colw):
                        # exp both heads, then their av contributions
                        atA = attn_pool.tile([128, FILL], BF16, name="atA", tag="at")
                        atB = attn_pool.tile([128, FILL], BF16, name="atB", tag="at")
                        nc.scalar.activation(atA[:, 0:colw], scA[:, 0:colw], Exp)
                        nc.scalar.activation(atB[:, 0:colw], scB[:, 0:colw], Exp)
                        for j, ks, off in fill_js:
                            for a in range(2):
                                h = 2 * p + a
                                lhs = vv[j][:, 65 * h:65 * h + 65]
                                tiles = avA if a == 0 else avB
                                at = atA if a == 0 else atB
                                for idx, i in enumerate(ks):
                                    nc.tensor.matmul(
                                        av_region(tiles, i),
                                        lhs,
                                        at[:, off + idx * 128: off + (idx + 1) * 128],
                                        start=False,
                                        stop=(j == last_j[i]),
                                        skip_group_check=True,
                                    )

                    fill_js = []
                    scA = scB = None
                    colw = 0
                    for j in range(NB):
                        ks = [i for i in col_kept[j] if i in irange]
                        if not ks:
                            continue
                        n = len(ks)
                        if scA is None or colw + n * 128 > FILL:
                            if scA is not None:
                                flush(fill_js, scA, scB, colw)
                            scA = scA_pool.tile([128, FILL], F32, name="scA", tag="scA")
                            scB = scB_pool.tile([128, FILL], F32, name="scB", tag="scB")
                            fill_js = []
                            colw = 0
                        # head A scores into scA, head B into scB (separate banks:
                        # concurrently-executing row-group-packed matmuls must not
                        # share a PSUM bank)
                        for a in range(2):
                            rows = slice(0, 64) if a == 0 else slice(64, 128)
                            dst = scA if a == 0 else scB
                            for run in _runs_of(ks):
                                idx0 = ks.index(run[0])
                                col = colw + idx0 * 128
                                width = len(run) * 128
                                qcol = run[0] * 128
                                done = 0
                                while done < width:
                                    seg = min(width - done, 512 - ((col + done) % 512))
                                    nc.tensor.matmul(
                                        dst[:, col + done: col + done + seg],
                                        kT[p][rows, j * 128:(j + 1) * 128],
                                        qT[p][rows, qcol + done: qcol + done + seg],
                                    )
                                    done += seg
                        fill_js.append((j, ks, colw))
                        colw += n * 128
                    if scA is not None:
                        flush(fill_js, scA, scB, colw)

                    # ---- normalization for this chunk ----
                    for i in irange:
                        icols = slice(i * 128, (i + 1) * 128)
                        stA = norm_pool.tile([65, 128], F32, name="stA", tag="stA")
                        nc.vector.tensor_copy(stA[:], av_region(avA, i))
                        stB = norm_pool.tile([65, 128], F32, name="stB", tag="stB")
                        nc.vector.tensor_copy(stB[:], av_region(avB, i))
                        rcA = norm_pool.tile([65, 128], F32, name="rcA", tag="rcA")
                        nc.vector.reciprocal(rcA[64:65, :], stA[64:65, :])
                        rcB = norm_pool.tile([65, 128], F32, name="rcB", tag="rcB")
                        nc.vector.reciprocal(rcB[64:65, :], stB[64:65, :])
                        bc = norm_pool.tile([128, 128], F32, name="bc", tag="bc")
                        nc.sync.dma_start(bc[0:64, :], _bcast_ap(rcA[64:65, :], 64))
                        nc.sync.dma_start(bc[64:128, :], _bcast_ap(rcB[64:65, :], 64))
                        stB2 = norm_pool.tile([128, 128], F32, name="stB2", tag="stB2")
                        nc.sync.dma_start(stB2[64:128, :], stB[0:64, :])
                        nc.vector.tensor_mul(outTbf[p][0:64, icols], stA[0:64, :], bc[0:64, :])
                        nc.vector.tensor_mul(outTbf[p][64:128, icols], stB2[64:128, :], bc[64:128, :])

        if "final" not in phases:
            return
        # ---- Phase 3: output projection (partial over this core's d-slice) -------
        with ExitStack() as fctx:
            fin_ps = fctx.enter_context(tc.tile_pool(name="fin_ps", bufs=4, space="PSUM"))
            fin_sb = fctx.enter_context(tc.tile_pool(name="fin_sb", bufs=4))
            for m in range(S // 128):
                pss = [fin_ps.tile([128, 512], F32, name="finps", tag="fin") for _ in range(2)]
                for p in range(2):
                    for n in range(2):
                        nc.tensor.matmul(
                            pss[n][:],
                            outTbf[p][:, m * 128:(m + 1) * 128],
                            wo[p][:, n * 512:(n + 1) * 512],
                            start=(p == 0),
                            stop=(p == 1),
                        )
                for n in range(2):
                    st = fin_sb.tile([128, 512], F32, name="finst", tag="finsb")
                    if (m + n) % 2 == 0:
                        nc.scalar.copy(st[:], pss[n][:])
                    else:
                        nc.vector.tensor_copy(st[:], pss[n][:])
                    nc.sync.dma_start(
                        outp_ap[m * 128:(m + 1) * 128, n * 512:(n + 1) * 512], st[:]
                    )


def _get_nc(kept, loop_n=0, phases=ALL_PHASES):
    key = (kept, loop_n, tuple(phases))
    if key in _nc_cache:
        return _nc_cache[key]
    nc = bacc.Bacc("TRN2", target_bir_lowering=False, debug=False, num_devices=NCORES)
    xT_ap = nc.dram_tensor("xT", [D, S], BF16, kind="ExternalInput").ap()
    wqT_ap = nc.dram_tensor("wqT", [D, E], BF16, kind="ExternalInput").ap()
    wkT_ap = nc.dram_tensor("wkT", [D, E], BF16, kind="ExternalInput").ap()
    wvT_ap = nc.dram_tensor("wvT", [D, E], BF16, kind="ExternalInput").ap()
    woT_ap = nc.dram_tensor("woT", [E, D], BF16, kind="ExternalInput").ap()
    outp_ap = nc.dram_tensor("outp", [S, D], F32, kind="ExternalOutput").ap()
    with tile.TileContext(nc) as tc:
        _emit(tc, (xT_ap, wqT_ap, wkT_ap, wvT_ap, woT_ap, outp_ap), kept,
              loop_n=loop_n, phases=phases)
    nc.compile()
    _nc_cache[key] = nc
    return nc


def kernel(x, Wq, Wk, Wv, Wo, bo, block_mask):
    x = np.asarray(x, dtype=np.float32)
    Wq = np.asarray(Wq, dtype=np.float32)
    Wk = np.asarray(Wk, dtype=np.float32)
    Wv = np.asarray(Wv, dtype=np.float32)
    Wo = np.asarray(Wo, dtype=np.float32)
    bo = np.asarray(bo, dtype=np.float32)
    mask = np.asarray(block_mask).astype(bool)

    kept = tuple(tuple(int(j) for j in np.nonzero(mask[i])[0]) for i in range(NB))
    assert all(len(js) > 0 for js in kept), "a query block row has no kept blocks"

    t0 = time.monotonic()
    nc = _get_nc(kept)
    t_compile = time.monotonic() - t0

    xT_b = [np.ascontiguousarray(x[b].T).astype(bf16) for b in range(B)]
    in_maps = []
    for c in range(NCORES):
        b = c // (NCORES // B)
        hs = c % (NCORES // B)
        sl = slice(hs * E, (hs + 1) * E)
        in_maps.append({
            "xT": xT_b[b],
            "wqT": np.ascontiguousarray((Wq[sl, :] / np.sqrt(np.float32(DH))).T).astype(bf16),
            "wkT": np.ascontiguousarray(Wk[sl, :].T).astype(bf16),
            "wvT": np.ascontiguousarray(Wv[sl, :].T).astype(bf16),
            "woT": np.ascontiguousarray(Wo[:, sl].T).astype(bf16),
        })

    t0 = time.monotonic()
    res = run_bass_kernel_spmd(nc, in_maps, list(range(NCORES)))
    t_run = time.monotonic() - t0

    out = np.zeros((B, S, D), np.float32)
    for c in range(NCORES):
        out[c // (NCORES // B)] += res.results[c]["outp"]
    out += bo[None, None, :]

    last_run_info.update(compile_s=t_compile, run_s=t_run, nc=nc)
    return out



# revision 31
# speedup vs baseline: 1.5972x; 1.5972x over previous
"""Block-sparse attention on 8 Trainium2 NeuronCores (Bass/Tile SPMD kernel).

Sharding: batch*head_groups across the 8 cores. Core c handles batch c//4 and
heads [4*(c%4), 4*(c%4)+4). Projection weights are sliced per core host-side
(pre-transposed + bf16-cast); the [16,16] block mask specializes the compiled
program (only kept blocks are computed). Each core emits a partial output
(its 256-wide d-slice pushed through Wo, stored bf16); the host sums the 4
partials per batch and adds the bias.

Per-core schedule (v3):
  - q^T/k^T pair-0 projections up front (PE), with their PSUM->SBUF copies on
    the scalar engine (idle until the first exp).
  - attention sweeps (pair, chunk-of-4 query blocks): scores for both heads
    into one [128, 1024] PSUM tile (head A bank 1, head B bank 2), one exp
    activation per flush, then AV with the exp'd tile as the matmul
    *stationary* so the output lands in natural [q, d] orientation at 65
    output columns per block (64 v-dims + denominator from a ones column).
  - v projections and pair-1 q^T/k^T projections and the final output
    projection run as *filler generators*, paced between flushes to keep the
    tensor engine busy while the activation engine streams exps; v blocks are
    force-drained just before the first AV that consumes them.
  - normalization: one reciprocal + one tensor_mul per (av tile, pair)
    (denominator is PSUM column 64 of each 65-col block -> per-partition
    scalar, broadcast along free axis via stride-0 AP), eager after the
    flush that completes the tile; out^T via bf16 XBAR DMA transpose.
  - final projection accumulates in PSUM, copied to bf16, stored via SWDGE
    (gpsimd) + SP queues; host sums partials in f32.
"""

import time
from collections import deque
from contextlib import ExitStack

import ml_dtypes
import numpy as np

import concourse.bass as bass
import concourse.tile as tile
from concourse import bacc, masks, mybir
from concourse.ap import AP as APClass
from concourse.bass_utils import run_bass_kernel_spmd

BF16 = mybir.dt.bfloat16
F32 = mybir.dt.float32
FP8 = mybir.dt.float8e4
bf16 = ml_dtypes.bfloat16
fp8 = ml_dtypes.float8_e4m3

# q/k projections run as fp8e4m3 DoubleRow matmuls (2 contraction chunks per
# pass). Weights are pre-scaled x256 host-side (e4m3 normals start at 2^-6;
# the raw 0.02-std weights would land subnormal) and rescaled by 1/256 in the
# PSUM->SBUF copy.
FP8_QK = True
W8SCALE = 256.0

B, S, D, H = 2, 2048, 1024, 16
DH = 64
BLK = 128
NB = 16
NCORES = 8
HPC = H // (NCORES // B)   # 4 heads per core
E = HPC * DH               # 256 projection columns per core
KD = D // 128              # 8 contraction chunks
CHUNK = 4                  # query blocks per attention chunk
FLUSH_PAIRS = 4            # (j,i) pairs per score flush (512 cols per head)

_nc_cache: dict = {}
last_run_info: dict = {}


def _ap3(sl, ap):
    """Raw AP constructor on the underlying tensor of a slice."""
    return APClass(sl.tensor, sl.offset, ap)


def _emit_body(tc, aps, kept):
    nc = tc.nc
    xT_ap, wqT_ap, wkT_ap, wvT_ap, woT_ap, outp_ap = aps
    Exp = mybir.ActivationFunctionType.Exp

    first_j = {i: kept[i][0] for i in range(NB)}
    last_j = {i: kept[i][-1] for i in range(NB)}
    col_kept = [[i for i in range(NB) if j in kept[i]] for j in range(NB)]

    with ExitStack() as ctx:
        persist = ctx.enter_context(tc.tile_pool(name="persist", bufs=1))

        # ---------------- persistent SBUF tiles ----------------
        xT = [persist.tile([128, S], BF16, name=f"xT{kd}", tag=f"xT{kd}")
              for kd in range(KD)]
        wv = persist.tile([128, KD * E], BF16, name="wv", tag="wv")
        if FP8_QK:
            # fp8 x pair tiles: xp[t][r, (two, s)] = x^T[256t + 128*two + r, s]
            xp = [persist.tile([128, 2 * S], FP8, name=f"xp{t}", tag=f"xp{t}")
                  for t in range(4)]
            wq = persist.tile([128, KD * E], FP8, name="wq", tag="wq")
            wk = persist.tile([128, KD * E], FP8, name="wk", tag="wk")
        else:
            wq = persist.tile([128, KD * E], BF16, name="wq", tag="wq")
            wk = persist.tile([128, KD * E], BF16, name="wk", tag="wk")
        wo = [persist.tile([128, D], BF16, name=f"wo{p}", tag=f"wo{p}")
              for p in range(2)]
        qT = [persist.tile([128, S], BF16, name=f"qT{p}", tag=f"qT{p}") for p in range(2)]
        kT = [persist.tile([128, S], BF16, name=f"kT{p}", tag=f"kT{p}") for p in range(2)]
        # vE: per s-block j, 4 heads of [v_h | 1] (65 cols each)
        vE = persist.tile([128, NB * HPC * 65], BF16, name="vE", tag="vE")
        outN = persist.tile([128, NB * 256], BF16, name="outN", tag="outN")
        outTbf = [persist.tile([128, S], BF16, name=f"oT{p}", tag=f"oT{p}")
                  for p in range(2)]

        # ---------------- input loads (spread across queues) ----------------
        def load_w(dst, src_ap, eng):
            # [1024, 256] dram -> [128, (kd, 256)] sbuf in ONE dma
            src = _ap3(src_ap, [[E, 128], [128 * E, KD], [1, E]])
            eng.dma_start(dst[:].rearrange("p (k e) -> p k e", k=KD), src)

        # x loads split into column quarters: projection group sc consumes
        # exactly x^T cols [sc*512, (sc+1)*512), so group sc0 can start after
        # only a quarter of x has landed
        if FP8_QK:
            # fp8 weights + x pair-tile halves first (q/k path), then bf16 x
            # for the v path. Activation queue stays clean for the exp stream.
            load_w(wq, wq8_ap, nc.sync)
            load_w(wk, wk8_ap, nc.sync)
            for half in range(2):
                lo = half * 1024
                for t in range(4):
                    src = _ap3(xp8_ap, [[S, 128], [128 * S, 2], [1, 1024]])
                    src = APClass(src.tensor, 256 * t * S + lo, src.ap)
                    nc.sync.dma_start(
                        xp[t][:].rearrange("p (two s) -> p two s", two=2)[:, :, lo:lo + 1024],
                        src)
                if half == 0:
                    load_w(wv, wvT_ap, nc.gpsimd)
            for half in range(2):
                lo = half * 1024
                for kd in range(KD):
                    eng = nc.sync if kd % 2 == 0 else nc.gpsimd
                    eng.dma_start(xT[kd][:, lo:lo + 1024],
                                  xT_ap[kd * 128:(kd + 1) * 128, lo:lo + 1024])
                if half == 0:
                    for p in range(2):
                        nc.gpsimd.dma_start(wo[p][:], woT_ap[p * 128:(p + 1) * 128, :])
        else:
            load_w(wq, wqT_ap, nc.sync)
            for quarter in range(4):
                lo = quarter * 512
                eng = nc.sync if quarter < 2 else nc.gpsimd
                for kd in range(KD):
                    eng.dma_start(xT[kd][:, lo:lo + 512],
                                  xT_ap[kd * 128:(kd + 1) * 128, lo:lo + 512])
                if quarter == 0:
                    load_w(wk, wkT_ap, nc.sync)
                    load_w(wv, wvT_ap, nc.gpsimd)
                elif quarter == 1:
                    for p in range(2):
                        nc.gpsimd.dma_start(wo[p][:], woT_ap[p * 128:(p + 1) * 128, :])

        # ones columns of vE (col 64 of each 65-block)
        vE4 = vE[:].rearrange("p (j h c) -> p (j h) c", j=NB, h=HPC)
        nc.gpsimd.memset(vE4[:, :, 64:65], 1.0)
        ident = persist.tile([128, 128], BF16, name="ident", tag="ident")
        masks.make_identity(nc, ident[:])

        # ---------------- pools ----------------
        sc_pool = ctx.enter_context(tc.tile_pool(name="sc_ps", bufs=2, space="PSUM"))
        fill_ps = ctx.enter_context(tc.tile_pool(name="fill_ps", bufs=2, space="PSUM"))
        av_pool = ctx.enter_context(tc.tile_pool(name="av_ps", bufs=1, space="PSUM"))
        at_pool = ctx.enter_context(tc.tile_pool(name="at_sb", bufs=3))
        norm_pool = ctx.enter_context(tc.tile_pool(name="norm_sb", bufs=8))
        fin_sb = ctx.enter_context(tc.tile_pool(name="fin_sb", bufs=4))

        # ---------------- projection emitters ----------------
        def _qk_matmuls(ps, w, p, sc):
            """yields cycle counts; accumulates the (p, sc) projection group"""
            if FP8_QK:
                w3 = w[:].rearrange("p (k e) -> p k e", k=KD)
                for t in range(4):
                    nc.tensor.matmul(
                        ps[:],
                        w3[:, 2 * t:2 * t + 2, p * 128:(p + 1) * 128],
                        xp[t][:].rearrange("p (two s) -> p two s", two=2)
                             [:, :, sc * 512:(sc + 1) * 512],
                        start=(t == 0), stop=(t == 3),
                        perf_mode=mybir.MatmulPerfMode.DoubleRow,
                    )
                    if t % 2 == 1:
                        yield 512
            else:
                for kd in range(KD):
                    nc.tensor.matmul(
                        ps[:],
                        w[:, kd * E + p * 128: kd * E + (p + 1) * 128],
                        xT[kd][:, sc * 512:(sc + 1) * 512],
                        start=(kd == 0), stop=(kd == KD - 1),
                    )
                    if kd % 2 == 1:
                        yield 1024

        def emit_qk_group(dst, w, p, sc, copy_eng):
            ps = fill_ps.tile([128, 512], F32, name="qkps", tag="fill")
            for _ in _qk_matmuls(ps, w, p, sc):
                pass
            dsl = dst[:, sc * 512:(sc + 1) * 512]
            if FP8_QK:
                nc.scalar.mul(dsl, ps[:], 1.0 / W8SCALE)
            else:
                nc.scalar.copy(dsl, ps[:])

        def gen_qk_group(dst, w, p, sc):
            ps = fill_ps.tile([128, 512], F32, name="qkps", tag="fill")
            yield from _qk_matmuls(ps, w, p, sc)
            dsl = dst[:, sc * 512:(sc + 1) * 512]
            if FP8_QK:
                nc.vector.tensor_scalar_mul(dsl, ps[:], 1.0 / W8SCALE)
            else:
                nc.vector.tensor_copy(dsl, ps[:])
            yield 0

        v_done = [0]

        def gen_v_block(m):
            # natural-orientation v for s-block m
            ps = fill_ps.tile([128, 256], F32, name="vps", tag="fill")
            for kd in range(KD):
                nc.tensor.matmul(
                    ps[:],
                    xT[kd][:, m * 128:(m + 1) * 128],
                    wv[:, kd * E:(kd + 1) * E],
                    start=(kd == 0), stop=(kd == KD - 1),
                )
                if kd % 4 == 3:
                    yield 1024
            ps2 = ps[:].rearrange("p (h c) -> p h c", h=HPC)
            nc.vector.tensor_copy(vE4[:, m * HPC:(m + 1) * HPC, 0:64], ps2)
            v_done[0] = m + 1

        def gen_fin(m):
            for n in range(2):
                ps = fill_ps.tile([128, 512], F32, name="finps", tag="fill")
                for p in range(2):
                    nc.tensor.matmul(
                        ps[:],
                        outTbf[p][:, m * 128:(m + 1) * 128],
                        wo[p][:, n * 512:(n + 1) * 512],
                        start=(p == 0), stop=(p == 1),
                        skip_group_check=True,
                    )
                st = fin_sb.tile([128, 512], BF16, name="finst", tag="finsb")
                nc.vector.tensor_copy(st[:], ps[:])
                nc.gpsimd.dma_start(
                    outp_ap[m * 128:(m + 1) * 128, n * 512:(n + 1) * 512], st[:])
                yield 1024

        # ---------------- filler machinery (need-keyed, paced) ----------------
        filler = {}          # key -> generator; dict preserves insertion order
        order = deque()
        pe_debt = [0]

        def add_filler(key, gen):
            filler[key] = gen
            order.append(key)

        def pull_filler(budget):
            pe_debt[0] = min(pe_debt[0] + budget, 3000)
            while order and pe_debt[0] > 0:
                k = order[0]
                gen = filler.get(k)
                if gen is None:
                    order.popleft()
                    continue
                try:
                    pe_debt[0] -= next(gen)
                except StopIteration:
                    del filler[k]
                    order.popleft()

        def drain_key(key):
            gen = filler.pop(key, None)
            if gen is not None:
                for _ in gen:
                    pass

        def drain_all():
            for k in list(filler):
                drain_key(k)

        # ---------------- upfront: first q0/k0 groups ----------------
        emit_qk_group(qT[0], wq, 0, 0, nc.scalar)
        emit_qk_group(kT[0], wk, 0, 0, nc.scalar)
        qk_done = {("q", 0, 0), ("k", 0, 0)}
        # remaining p0 groups + v + p1 groups become paced/need-driven filler
        for sc in (1, 2, 3):
            add_filler(("k", 0, sc), gen_qk_group(kT[0], wk, 0, sc))
        for sc in (1, 2, 3):
            add_filler(("q", 0, sc), gen_qk_group(qT[0], wq, 0, sc))
        for m in range(NB):
            add_filler(("v", m), gen_v_block(m))
        for key, dst, w in (("q", qT, wq), ("k", kT, wk)):
            for sc in range(4):
                add_filler((key, 1, sc), gen_qk_group(dst[1], w, 1, sc))

        # ---------------- attention ----------------
        def norm_pair(p, c, g, av_t):
            """normalize both query blocks of av tile g; transpose; queue fin."""
            av4 = av_t[:, 0:260].rearrange("p (i h c) -> p i h c", i=2, h=2)
            rc = norm_pool.tile([128, 4], F32, name="rc", tag="rc")
            rc3 = rc[:].rearrange("p (a b c) -> p a b c", a=2, b=2)
            nc.vector.reciprocal(rc3, av4[:, :, :, 64:65])
            rcap = rc[:]
            cs = list(rcap.ap[1])[0]
            rc_b = _ap3(rcap, [list(rcap.ap[0]), [cs * 2, 2], [cs, 2], [0, 64]])
            i0 = c * CHUNK + 2 * g
            dst = _ap3(
                outN[:, i0 * 256 + p * 128: i0 * 256 + (p + 1) * 128],
                [list(outN[:].ap[0]), [256, 2], [64, 2], [1, 64]],
            )
            nc.vector.tensor_mul(dst, av4[:, :, :, 0:64], rc_b)
            for i in (i0, i0 + 1):
                # PE transpose (fast, low-latency) into a fill-pool PSUM slot,
                # then DVE copy into the bf16 out^T tile
                tp = fill_ps.tile([128, 128], BF16, name="trps", tag="fill")
                nc.tensor.transpose(
                    tp[:],
                    outN[:, i * 256 + p * 128: i * 256 + (p + 1) * 128],
                    ident[:],
                )
                nc.vector.tensor_copy(outTbf[p][:, i * 128:(i + 1) * 128], tp[:])
                if p == 1:
                    add_filler(("f", i), gen_fin(i))

        AV_DEFER = 2
        for p in range(2):
            for c in range(NB // CHUNK):
                irange = range(c * CHUNK, (c + 1) * CHUNK)
                # full-bank tiles: start=True lazily zeroes the whole 2KB
                # PSUM zero-region, so regions sharing a bank must issue
                # exactly one start and let first-touch auto-zero the rest
                av_tiles = [
                    av_pool.tile([128, 512], F32, name=f"av{g}", tag=f"av{g}")
                    for g in range(2)
                ]
                av_started = [False, False]
                # last (j,i) pair index per av tile, for eager norm
                pairs = []
                for j in range(NB):
                    for i in col_kept[j]:
                        if i in irange:
                            pairs.append((j, i))
                last_pair_of_tile = {}
                for idx, (j, i) in enumerate(pairs):
                    last_pair_of_tile[(i % CHUNK) // 2] = idx

                nflush = (len(pairs) + FLUSH_PAIRS - 1) // FLUSH_PAIRS
                pending = deque()   # (emit_av, needed_js, idx_end)
                normed = {0: False, 1: False}

                def flush_one_pending():
                    emit_av, js, idx_end = pending.popleft()
                    for j in js:
                        drain_key(("v", j))
                    emit_av()
                    for g in (0, 1):
                        if (g in last_pair_of_tile and not normed[g]
                                and last_pair_of_tile[g] <= idx_end):
                            norm_pair(p, c, g, av_tiles[g])
                            normed[g] = True

                for f in range(nflush):
                    fp = pairs[f * FLUSH_PAIRS:(f + 1) * FLUSH_PAIRS]
                    # prereq projections for this flush's scores
                    drain_key(("q", p, c))
                    for j in {j for j, _ in fp}:
                        drain_key(("k", p, j // 4))
                    pull_filler(500)
                    sc_t = sc_pool.tile([128, 1024], F32, name="sc", tag="sc")
                    pos = 0
                    while pos < len(fp):
                        j, i0 = fp[pos]
                        ln = 1
                        while (pos + ln < len(fp) and fp[pos + ln][0] == j
                               and fp[pos + ln][1] == fp[pos + ln - 1][1] + 1):
                            ln += 1
                        for a in range(2):
                            rows = slice(0, 64) if a == 0 else slice(64, 128)
                            nc.tensor.matmul(
                                sc_t[:, a * 512 + pos * 128: a * 512 + (pos + ln) * 128],
                                kT[p][rows, j * 128:(j + 1) * 128],
                                qT[p][rows, i0 * 128:(i0 + ln) * 128],
                            )
                        pos += ln

                    # one exp for both heads
                    at_t = at_pool.tile([128, 1024], BF16, name="at", tag="at")
                    w = len(fp) * 128
                    if w == 512:
                        nc.scalar.activation(at_t[:], sc_t[:], Exp)
                    else:
                        sc3 = sc_t[:].rearrange("p (a c) -> p a c", a=2)
                        at3 = at_t[:].rearrange("p (a c) -> p a c", a=2)
                        nc.scalar.activation(at3[:, :, 0:w], sc3[:, :, 0:w], Exp)

                    pull_filler(600)

                    def make_av(fp=fp, at_t=at_t):
                        def emit():
                            for idx, (j, i) in enumerate(fp):
                                li = i % CHUNK
                                g = li // 2
                                av_t = av_tiles[g]
                                base = (li % 2) * 130
                                for a in range(2):
                                    h = 2 * p + a
                                    nc.tensor.matmul(
                                        av_t[:, base + a * 65: base + (a + 1) * 65],
                                        at_t[:, a * 512 + idx * 128: a * 512 + (idx + 1) * 128],
                                        vE[:, (j * HPC + h) * 65:(j * HPC + h + 1) * 65],
                                        start=not av_started[g],
                                        stop=(j == last_j[i]),
                                        skip_group_check=True,
                                    )
                                    av_started[g] = True
                        return emit
                    pending.append((make_av(), sorted({j for j, _ in fp}),
                                    (f + 1) * FLUSH_PAIRS - 1))
                    if len(pending) > AV_DEFER:
                        flush_one_pending()
                while pending:
                    flush_one_pending()
        drain_all()


def _get_nc(kept):
    key = kept
    if key in _nc_cache:
        return _nc_cache[key]
    nc = bacc.Bacc("TRN2", target_bir_lowering=False, debug=False, num_devices=NCORES)
    xT_ap = nc.dram_tensor("xT", [D, S], BF16, kind="ExternalInput").ap()
    wqT_ap = nc.dram_tensor("wqT", [D, E], BF16, kind="ExternalInput").ap()
    wkT_ap = nc.dram_tensor("wkT", [D, E], BF16, kind="ExternalInput").ap()
    wvT_ap = nc.dram_tensor("wvT", [D, E], BF16, kind="ExternalInput").ap()
    woT_ap = nc.dram_tensor("woT", [E, D], BF16, kind="ExternalInput").ap()
    outp_ap = nc.dram_tensor("outp", [S, D], BF16, kind="ExternalOutput").ap()
    with tile.TileContext(nc) as tc:
        _emit_body(tc, (xT_ap, wqT_ap, wkT_ap, wvT_ap, woT_ap, outp_ap), kept)
    nc.compile()
    _nc_cache[key] = nc
    return nc


def kernel(x, Wq, Wk, Wv, Wo, bo, block_mask):
    x = np.asarray(x, dtype=np.float32)
    Wq = np.asarray(Wq, dtype=np.float32)
    Wk = np.asarray(Wk, dtype=np.float32)
    Wv = np.asarray(Wv, dtype=np.float32)
    Wo = np.asarray(Wo, dtype=np.float32)
    bo = np.asarray(bo, dtype=np.float32)
    mask = np.asarray(block_mask).astype(bool)

    kept = tuple(tuple(int(j) for j in np.nonzero(mask[i])[0]) for i in range(NB))
    assert all(len(js) > 0 for js in kept), "a query block row has no kept blocks"

    t0 = time.monotonic()
    nc = _get_nc(kept)
    t_compile = time.monotonic() - t0

    xT_b = [np.ascontiguousarray(x[b].T).astype(bf16) for b in range(B)]
    in_maps = []
    for c in range(NCORES):
        b = c // (NCORES // B)
        hs = c % (NCORES // B)
        sl = slice(hs * E, (hs + 1) * E)
        in_maps.append({
            "xT": xT_b[b],
            "wqT": np.ascontiguousarray((Wq[sl, :] / np.sqrt(np.float32(DH))).T).astype(bf16),
            "wkT": np.ascontiguousarray(Wk[sl, :].T).astype(bf16),
            "wvT": np.ascontiguousarray(Wv[sl, :].T).astype(bf16),
            "woT": np.ascontiguousarray(Wo[:, sl].T).astype(bf16),
        })

    t0 = time.monotonic()
    res = run_bass_kernel_spmd(nc, in_maps, list(range(NCORES)))
    t_run = time.monotonic() - t0

    out = np.zeros((B, S, D), np.float32)
    for c in range(NCORES):
        out[c // (NCORES // B)] += res.results[c]["outp"].astype(np.float32)
    out += bo[None, None, :]

    last_run_info.update(compile_s=t_compile, run_s=t_run, nc=nc)
    return out
